# revision 8
# baseline (speedup 1.0000x reference)
"""Trainium2 8-core attention kernel (B=2, N=2048, D=1024, H=16).

Sharding: core c = 4*b + g handles batch b, query rows [g*512, (g+1)*512),
all 16 heads. Each core receives the full x^T of its batch with sequence
blocks rotated so its own block sits at column 0 (keys are permutation-
invariant under softmax). Heads 0-3 compute K/V locally over the whole
sequence; the remaining heads' K/V shards AllGather in three 4-rank
chunks that overlap attention.

Schedule (v2): DMA streams are k-interleaved and stage-1 runs k-outer
across PSUM banks so the PE starts as soon as the first x^T/W tiles
land. V for m-blocks 0-3 is computed over all 8 duos at once (N=1024),
which both feeds the local duos' first k-tiles and stages every AG
chunk's V payload up front - all three AllGathers are in flight within
a few microseconds of each other, long before their consumer duos.
Leftover stage-1 work (V m4-15 for the local duos, chunk K computes,
Q tiles 4-7, W_proj loads) is emitted as fillers inside the exp-paced
attention window. Attention runs in transposed-score orientation with
row-tiled concurrent score matmuls and a ones-column on V accumulating
softmax denominators in PSUM row 64. Normalization is PE-free:
reciprocal_approx_fast (DVE) + partition_broadcast (GpSimd) + one DVE
multiply. The projection's first PSUM groups pre-run their k<7 steps
inside duo 7's window; output stores stream per block.
"""

import sys

if "/opt/trn_rl_repo" not in sys.path:
    sys.path.insert(0, "/opt/trn_rl_repo")

import numpy as np
import ml_dtypes

import concourse.bass as bass
import concourse.mybir as mybir
from concourse import bacc, tile
from concourse import bass_utils

FP32 = mybir.dt.float32
BF16 = mybir.dt.bfloat16

B, N, D = 2, 2048, 1024
H, HD = 16, 64
SCALE = HD ** -0.5
NC = 8
GROUPS = [[0, 1, 2, 3], [4, 5, 6, 7]]
NQ = N // 4          # query rows per core (512)
KT = N // 128        # key k-tiles (16)
CT = D // 128        # 128-channel tiles per D (8)
LOCAL_DUOS = 2                       # duos computed locally over full seq
CHUNKS = [2, 2, 2]                   # AllGather chunks (duos), for duos 2..7
DUO_ELEMS = 128 * NQ + 2 * NQ * HD   # per-duo: 1 K^T pair + 2 V heads
CH_OFF = [sum(CHUNKS[:i]) for i in range(len(CHUNKS) + 1)]  # AG duo offsets

_compiled = None


def build():
    from contextlib import ExitStack

    nc = bacc.Bacc("TRN2", target_bir_lowering=False, debug=False, num_devices=NC)

    xT = nc.dram_tensor("xT", [D, N], BF16, kind="ExternalInput")
    w_qkv = nc.dram_tensor("w_qkv", [D, 3 * D], BF16, kind="ExternalInput")
    w_proj = nc.dram_tensor("w_proj", [D, D], BF16, kind="ExternalInput")
    b_qk = nc.dram_tensor("b_qk", [128, 16], FP32, kind="ExternalInput")
    b_v = nc.dram_tensor("b_v", [128, D], FP32, kind="ExternalInput")
    b_prj = nc.dram_tensor("b_prj", [128, D], FP32, kind="ExternalInput")
    out = nc.dram_tensor("out", [NQ, D], FP32, kind="ExternalOutput")

    with tile.TileContext(nc) as tc, ExitStack() as ctx:
        wqk_pool = ctx.enter_context(tc.tile_pool(name="wqk", bufs=16))
        wv_pool = ctx.enter_context(tc.tile_pool(name="wv", bufs=8))
        wp_pool = ctx.enter_context(tc.tile_pool(name="wp", bufs=8))
        xt_pool = ctx.enter_context(tc.tile_pool(name="xt", bufs=8))
        qt_pool = ctx.enter_context(tc.tile_pool(name="qt", bufs=8))
        bias_pool = ctx.enter_context(tc.tile_pool(name="bias", bufs=3))
        stg_pool = ctx.enter_context(tc.tile_pool(name="stg", bufs=3))
        vstg_pool = ctx.enter_context(tc.tile_pool(name="vstg", bufs=3))
        ktp_pool = ctx.enter_context(tc.tile_pool(name="ktp", bufs=4))
        vsb_pool = ctx.enter_context(tc.tile_pool(name="vsb", bufs=5))
        es_pool = ctx.enter_context(tc.tile_pool(name="es", bufs=6))
        ot_pool = ctx.enter_context(tc.tile_pool(name="ot", bufs=8))
        nrm_pool = ctx.enter_context(tc.tile_pool(name="nrm", bufs=3))
        y_pool = ctx.enter_context(tc.tile_pool(name="yy", bufs=3))
        # PSUM: psA = 2 x [128,1024] (4 banks), psB = 4 x [128,512] (4 banks)
        psA = ctx.enter_context(tc.tile_pool(name="psA", bufs=2, space="PSUM"))
        psB = ctx.enter_context(tc.tile_pool(name="psB", bufs=4, space="PSUM"))
        dram = ctx.enter_context(tc.tile_pool(name="dram", bufs=1, space="DRAM"))

        # ---- DMA streams: xt[k] + K-weight[k] interleaved, then Q-half0,
        # ---- then V weights, then Q-half1 ----
        xt = []
        wqk_k = []
        for k in range(CT):
            t = xt_pool.tile([128, N], BF16, tag="xt", name=f"xt{k}")
            nc.scalar.dma_start(t[:], xT.ap()[k * 128:(k + 1) * 128, :])
            xt.append(t)
            t = wqk_pool.tile([128, D], BF16, tag="wqk", name=f"wqkK{k}")
            nc.sync.dma_start(t[:], w_qkv.ap()[k * 128:(k + 1) * 128, D:2 * D])
            wqk_k.append(t)

        bqk_sb = bias_pool.tile([128, 16], FP32, tag="bias")
        nc.scalar.dma_start(bqk_sb[:], b_qk.ap()[:])
        bv_sb = bias_pool.tile([128, D], FP32, tag="bias")
        nc.scalar.dma_start(bv_sb[:], b_v.ap()[:])

        wqk_q = []
        for k in range(CT):
            t = wqk_pool.tile([128, D], BF16, tag="wqk", name=f"wqkQ{k}")
            nc.gpsimd.dma_start(t[:, 0:512], w_qkv.ap()[k * 128:(k + 1) * 128, 0:512])
            wqk_q.append(t)
        wv_t = []
        for k in range(CT):
            t = wv_pool.tile([128, D], BF16, tag="wv", name=f"wv{k}")
            nc.sync.dma_start(t[:], w_qkv.ap()[k * 128:(k + 1) * 128, 2 * D:3 * D])
            wv_t.append(t)
        for k in range(CT):
            nc.gpsimd.dma_start(
                wqk_q[k][:, 512:1024],
                w_qkv.ap()[k * 128:(k + 1) * 128, 512:1024],
            )

        # ---- DRAM bounce + AG buffers (layout identical to baseline) ----
        TOT = CH_OFF[-1] * DUO_ELEMS
        kv_in = dram.tile([TOT], BF16, tag="kvin")
        kv_ag = dram.tile([4 * TOT], BF16, tag="kvag")

        duo_tiles = {}

        # ---- phase 1: K^T for local duos 0,1 - k-outer over 8 PSUM groups
        # (psA tile halves hold (duo, r01) pairs; psB tiles hold r23) ----
        psK = {}
        for dd in range(2):
            a = psA.tile([128, 2 * NQ], FP32, tag="sq", name=f"psKa{dd}")
            b0 = psB.tile([128, NQ], FP32, tag="acc", name=f"psKb{dd}0")
            b1 = psB.tile([128, NQ], FP32, tag="acc", name=f"psKb{dd}1")
            psK[dd] = [a[:, 0:NQ], a[:, NQ:2 * NQ], b0[:], b1[:]]
        for k in range(CT):
            for dd in range(2):
                for r in range(4):
                    nc.tensor.matmul(
                        psK[dd][r], wqk_k[k][:, dd * 128:(dd + 1) * 128],
                        xt[k][:, r * NQ:(r + 1) * NQ],
                        start=(k == 0), stop=(k == CT - 1),
                    )
        ktp01 = []
        for dd in range(2):
            ktp = ktp_pool.tile([128, N], BF16, tag="ktp", name=f"ktpL{dd}")
            for r in range(4):
                nc.vector.tensor_scalar_add(
                    ktp[:, r * NQ:(r + 1) * NQ], psK[dd][r],
                    bqk_sb[:, 8 + dd:9 + dd],
                )
            ktp01.append(ktp)

        # local V tiles for duos 0,1 (filled progressively below)
        va_loc = []
        for dd in range(2):
            va = vsb_pool.tile([128, KT * 2 * (HD + 1)], BF16, tag="vsb",
                               name=f"vaL{dd}")
            nc.vector.memset(
                va[:].rearrange("cc (tj ef) -> cc tj ef", ef=HD + 1)[:, :, HD:HD + 1],
                1.0,
            )
            va_loc.append(va)
            duo_tiles[dd] = (ktp01[dd], va)

        # ---- phase 2: Q tiles 0-3 (psB) and V m0-1 over all duos (psA),
        # k-outer; then V m2-3, Q happens first so scores can start ----
        qt = [None] * CT

        def emit_q(t, ps):
            for k in range(CT):
                nc.tensor.matmul(
                    ps[:], wqk_q[k][:, t * 128:(t + 1) * 128], xt[k][:, 0:NQ],
                    start=(k == 0), stop=(k == CT - 1),
                )
            sb = qt_pool.tile([128, NQ], BF16, tag="qt", name=f"qt{t}")
            nc.vector.tensor_scalar_add(sb[:], ps[:], bqk_sb[:, t:t + 1])
            qt[t] = sb

        # V staging tiles per chunk: [p, m(4), h(2*nduo), e(64)]
        sbv_c = []
        for c, nduo in enumerate(CHUNKS):
            sbv_c.append(vstg_pool.tile([128, 4 * 128 * nduo], BF16, tag="vstg",
                                        name=f"vchunk{c}"))

        def drain_vm(m, ps):
            # ps: [128, 1024] = V for all 8 duos, m-block m. Local va writes
            # for duos 0,1 plus chunk staging for duos 2-7.
            for dd in range(LOCAL_DUOS):
                va4 = va_loc[dd][:].rearrange(
                    "cc (t j ef) -> cc t j ef", t=KT, j=2, ef=HD + 1
                )
                nc.vector.scalar_tensor_tensor(
                    va4[:, m, :, 0:HD],
                    ps[:].rearrange("p (h e) -> p h e", e=HD)[:, 2 * dd:2 * dd + 2, :],
                    0.0,
                    bv_sb[:].rearrange("p (h e) -> p h e", e=HD)[:, 2 * dd:2 * dd + 2, :],
                    op0=mybir.AluOpType.bypass, op1=mybir.AluOpType.add,
                )
            for c, nduo in enumerate(CHUNKS):
                d0 = LOCAL_DUOS + CH_OFF[c]
                nc.vector.scalar_tensor_tensor(
                    sbv_c[c][:, m * 128 * nduo:(m + 1) * 128 * nduo],
                    ps[:, d0 * 128:(d0 + nduo) * 128], 0.0,
                    bv_sb[:, d0 * 128:(d0 + nduo) * 128],
                    op0=mybir.AluOpType.bypass, op1=mybir.AluOpType.add,
                )

        # Q tiles 0,1 first (scores for duo 0/1), then V m0-1 k-outer
        qps = [psB.tile([128, NQ], FP32, tag="acc", name=f"psQ{t}")
               for t in range(2)]
        vps = [psA.tile([128, 2 * NQ], FP32, tag="sq", name=f"psVm{m}")
               for m in range(2)]
        for k in range(CT):
            for t in range(2):
                nc.tensor.matmul(
                    qps[t][:], wqk_q[k][:, t * 128:(t + 1) * 128], xt[k][:, 0:NQ],
                    start=(k == 0), stop=(k == CT - 1),
                )
            for m in range(2):
                for half in range(2):
                    nc.tensor.matmul(
                        vps[m][:, half * NQ:(half + 1) * NQ],
                        xt[k][:, m * 128:(m + 1) * 128],
                        wv_t[k][:, half * NQ:(half + 1) * NQ],
                        start=(k == 0), stop=(k == CT - 1),
                    )
        for t in range(2):
            sb = qt_pool.tile([128, NQ], BF16, tag="qt", name=f"qt{t}")
            nc.vector.tensor_scalar_add(sb[:], qps[t][:], bqk_sb[:, t:t + 1])
            qt[t] = sb
        for m in range(2):
            drain_vm(m, vps[m])

        # V m2-3 (k-inner; all wv present by now)
        for m in range(2, 4):
            ps = psA.tile([128, 2 * NQ], FP32, tag="sq", name=f"psVm{m}")
            for k in range(CT):
                for half in range(2):
                    nc.tensor.matmul(
                        ps[:, half * NQ:(half + 1) * NQ],
                        xt[k][:, m * 128:(m + 1) * 128],
                        wv_t[k][:, half * NQ:(half + 1) * NQ],
                        start=(k == 0), stop=(k == CT - 1),
                    )
            drain_vm(m, ps)

        # ---- chunk K computes + AG launches (chunk 0 now; 1,2 as fillers) ----
        def emit_chunk_k(c):
            nduo = CHUNKS[c]
            base = CH_OFF[c] * DUO_ELEMS
            d0 = LOCAL_DUOS + CH_OFF[c]
            ksz = nduo * 128 * NQ
            kin = kv_in[base:base + ksz].rearrange("(p q) -> p q", q=NQ)
            vin = kv_in[base + ksz:base + nduo * DUO_ELEMS].rearrange(
                "(p m h e) -> p m h e", p=128, m=4, h=2 * nduo, e=HD
            )
            for tt in range(nduo):
                d = d0 + tt
                ps = psB.tile([128, NQ], FP32, tag="acc", name=f"psKc{c}{tt}")
                for k in range(CT):
                    nc.tensor.matmul(
                        ps[:], wqk_k[k][:, d * 128:(d + 1) * 128],
                        xt[k][:, 0:NQ],
                        start=(k == 0), stop=(k == CT - 1),
                    )
                sb = stg_pool.tile([128, NQ], BF16, tag="stg", name=f"ksb{c}{tt}")
                nc.vector.tensor_scalar_add(sb[:], ps[:], bqk_sb[:, 8 + d:9 + d])
                nc.sync.dma_start(kin[tt * 128:(tt + 1) * 128, :], sb[:])
            nc.sync.dma_start(vin.rearrange("p m h e -> p (m h e)"), sbv_c[c][:])
            nc.gpsimd.collective_compute(
                "AllGather", mybir.AluOpType.bypass, replica_groups=GROUPS,
                ins=[kv_in[base:base + nduo * DUO_ELEMS].opt()],
                outs=[kv_ag[4 * base:4 * (base + nduo * DUO_ELEMS)].opt()],
            )

        def emit_chunk_loads(c):
            nduo = CHUNKS[c]
            base = CH_OFF[c] * DUO_ELEMS
            d0 = LOCAL_DUOS + CH_OFF[c]
            cbase4 = 4 * base
            blk = kv_ag[cbase4:cbase4 + 4 * nduo * DUO_ELEMS]
            for dd in range(nduo):
                d = d0 + dd
                ktp = ktp_pool.tile([128, N], BF16, tag="ktp", name=f"ktp{d}")
                nc.sync.dma_start(
                    ktp[:].rearrange("p (r q) -> p r q", r=4),
                    blk.rearrange("(r x p q) -> x p r q",
                                  r=4, x=nduo * DUO_ELEMS // (128 * NQ),
                                  p=128, q=NQ)[dd],
                )
                va = vsb_pool.tile([128, KT * 2 * (HD + 1)], BF16, tag="vsb",
                                   name=f"va{d}")
                va5 = va[:].rearrange(
                    "cc (r sh j ef) -> cc r sh j ef", r=4, sh=4, j=2, ef=HD + 1
                )
                for r in range(4):
                    src = bass.AP(
                        blk.tensor,
                        blk.offset + r * nduo * DUO_ELEMS + nduo * 128 * NQ
                        + 2 * dd * HD,
                        [[nduo * 512, 128], [128 * nduo, 4], [64, 2], [1, HD]],
                    )
                    eng = nc.gpsimd if r < 2 else nc.sync
                    eng.dma_start(va5[:, r, :, :, 0:HD], src)
                nc.vector.memset(
                    va[:].rearrange("cc (tj ef) -> cc tj ef",
                                    ef=HD + 1)[:, :, HD:HD + 1],
                    1.0,
                )
                duo_tiles[d] = (ktp, va)

        emit_chunk_k(0)
        emit_chunk_loads(0)

        # ---- filler units: emitted inside the attention window ----
        fillers = []

        def filler_q(t):
            def go():
                ps = psB.tile([128, NQ], FP32, tag="acc", name=f"psQ{t}")
                emit_q(t, ps)
            return go

        def filler_vloc(m):
            def go():
                ps = psB.tile([128, 2 * LOCAL_DUOS * HD], FP32, tag="acc",
                              name=f"psVL{m}")
                for k in range(CT):
                    nc.tensor.matmul(
                        ps[:], xt[k][:, m * 128:(m + 1) * 128],
                        wv_t[k][:, 0:2 * LOCAL_DUOS * HD],
                        start=(k == 0), stop=(k == CT - 1),
                    )
                for dd in range(LOCAL_DUOS):
                    va4 = va_loc[dd][:].rearrange(
                        "cc (t j ef) -> cc t j ef", t=KT, j=2, ef=HD + 1
                    )
                    nc.vector.scalar_tensor_tensor(
                        va4[:, m, :, 0:HD],
                        ps[:].rearrange("p (h e) -> p h e", e=HD)[:, 2 * dd:2 * dd + 2, :],
                        0.0,
                        bv_sb[:].rearrange("p (h e) -> p h e", e=HD)[:, 2 * dd:2 * dd + 2, :],
                        op0=mybir.AluOpType.bypass, op1=mybir.AluOpType.add,
                    )
            return go

        wp = []
        bp_sb = bias_pool.tile([128, D], FP32, tag="bias")

        def filler_wp(half):
            def go():
                if half == 0:
                    nc.sync.dma_start(bp_sb[:], b_prj.ap()[:])
                for k in range(half * 4, half * 4 + 4):
                    t = wp_pool.tile([128, D], BF16, tag="wp", name=f"wp{k}")
                    nc.sync.dma_start(t[:], w_proj.ap()[k * 128:(k + 1) * 128, :])
                    wp.append(t)
            return go

        def filler_chunk(c):
            def go():
                emit_chunk_k(c)
                emit_chunk_loads(c)
            return go

        # V m4-15 handled inline per-kt during duo 0; the rest as fillers.
        fillers.extend([filler_q(2), filler_q(3)])
        fillers.extend([filler_chunk(1)])
        fillers.extend([filler_q(4), filler_q(5)])
        fillers.extend([filler_chunk(2)])
        fillers.extend([filler_q(6), filler_q(7)])
        fillers.extend([filler_wp(0), filler_wp(1)])

        def pump(n):
            for _ in range(n):
                if fillers:
                    fillers.pop(0)()

        # ---- attention ----
        ot = []
        norm_q = []        # (ocs pair, duo) awaiting normalization

        def drain_oacc(o_acc, d):
            ocs = []
            for j in range(2):
                oc = nrm_pool.tile([HD + 1, NQ], FP32, tag="oc",
                                   name=f"oc{d}_{j}")
                nc.vector.tensor_copy(oc[:], o_acc[j][0:HD + 1, :])
                ocs.append(oc)
            return ocs

        def normalize(ocs, d):
            otd = ot_pool.tile([128, NQ], BF16, tag="ot", name=f"ot{d}")
            for j in range(2):
                # reciprocal_approx_fast mishandles base-partition-64 inputs;
                # stage the denominator row at partition 0 first.
                den0 = nrm_pool.tile([1, NQ], FP32, tag="den0", name=f"dn{d}_{j}",
                                     bufs=2)
                nc.vector.tensor_copy(den0[:], ocs[j][HD:HD + 1, :])
                rr = nrm_pool.tile([1, NQ], FP32, tag="rr", name=f"rr{d}_{j}",
                                   bufs=2)
                nc.vector.reciprocal_approx_fast(rr[:], den0[:])
                rb = nrm_pool.tile([HD, NQ], FP32, tag="rb", name=f"rb{d}_{j}",
                                   bufs=2)
                nc.gpsimd.partition_broadcast(rb[:], rr[:])
                nc.vector.scalar_tensor_tensor(
                    otd[j * HD:(j + 1) * HD, :],
                    ocs[j][0:HD, :], 0.0, rb[:],
                    op0=mybir.AluOpType.bypass, op1=mybir.AluOpType.mult,
                )
            ot.append(otd)

        for d in range(H // 2):
            ktp, va = duo_tiles[d]
            va4 = va[:].rearrange("cc (t j ef) -> cc t j ef", t=KT, j=2, ef=HD + 1)

            o_acc = [
                psB.tile([128, NQ], FP32, tag="acc", name=f"oacc{d}_{j}")
                for j in range(2)
            ]
            es_tiles = [None] * KT

            def emit_pv(kt_i):
                es_kt = es_tiles[kt_i]
                for j in range(2):
                    nc.tensor.matmul(
                        o_acc[j][0:HD + 1, :],
                        va4[:, kt_i, j, :],
                        es_kt[:, j * NQ:(j + 1) * NQ],
                        start=(kt_i == 0), stop=(kt_i == KT - 1),
                    )

            pv_lag = 3 if d == 0 else 1
            for kt in range(KT):
                # duo 0: V m-tiles 4-15 are produced just ahead of their PV
                if d == 0 and 2 <= kt < 14:
                    filler_vloc(kt + 2)()
                s = psA.tile([128, 2 * NQ], FP32, tag="sq", name=f"s{d}_{kt}")
                for i in range(2):
                    nc.tensor.matmul(
                        s[:, i * NQ:(i + 1) * NQ],
                        ktp[i * HD:(i + 1) * HD, kt * 128:(kt + 1) * 128],
                        qt[d][i * HD:(i + 1) * HD, :],
                        start=True, stop=True,
                    )
                es = es_pool.tile([128, 2 * NQ], BF16, tag="es",
                                  name=f"es{d}_{kt}")
                nc.scalar.activation(
                    es[:], s[:], mybir.ActivationFunctionType.Exp, scale=SCALE
                )
                es_tiles[kt] = es
                if kt >= pv_lag:
                    emit_pv(kt - pv_lag)
                if kt == 4 and norm_q:
                    normalize(*norm_q.pop(0))
                if kt == 8 and 1 <= d <= 5:
                    pump(1)
                elif kt == 12 and d <= 5:
                    pump(1)
            for kt_i in range(KT - pv_lag, KT):
                emit_pv(kt_i)
            norm_q.append((drain_oacc(o_acc, d), d))

        pump(len(fillers))

        # ---- projection: groups 0-1 pre-run k0-6 inside duo 7 via the
        # normal emission position; after last normalize, finish all ----
        while norm_q:
            normalize(*norm_q.pop(0))

        for m in range(NQ // 128):
            for n0 in range(D // 512):
                ps = psB.tile([128, 512], FP32, tag="acc", name=f"psP{m}{n0}")
                for k in range(CT):
                    nc.tensor.matmul(
                        ps[:], ot[k][:, m * 128:(m + 1) * 128],
                        wp[k][:, n0 * 512:(n0 + 1) * 512],
                        start=(k == 0), stop=(k == CT - 1),
                    )
                y = y_pool.tile([128, 512], FP32, tag="yy", name=f"y{m}{n0}")
                nc.vector.scalar_tensor_tensor(
                    y[:], ps[:], 0.0, bp_sb[:, n0 * 512:(n0 + 1) * 512],
                    op0=mybir.AluOpType.bypass, op1=mybir.AluOpType.add,
                )
                nc.sync.dma_start(
                    out.ap()[m * 128:(m + 1) * 128, n0 * 512:(n0 + 1) * 512], y[:]
                )

    nc.compile()
    return nc


def make_in_maps(x, W_qkv, b_qkv, W_proj, b_proj):
    x = np.asarray(x, dtype=np.float32)
    W_qkv = np.asarray(W_qkv, dtype=np.float32)
    b_qkv = np.asarray(b_qkv, dtype=np.float32)
    W_proj = np.asarray(W_proj, dtype=np.float32)
    b_proj = np.asarray(b_proj, dtype=np.float32)

    wq_bf = W_qkv.astype(ml_dtypes.bfloat16)
    wp_bf = W_proj.astype(ml_dtypes.bfloat16)
    bqk = np.ascontiguousarray(b_qkv[:2 * D].reshape(16, 128).T)
    bv = np.tile(b_qkv[2 * D:], (128, 1)).astype(np.float32)
    bp = np.tile(b_proj, (128, 1)).astype(np.float32)

    in_maps = []
    for c in range(NC):
        b, g = divmod(c, 4)
        xt_rot = np.concatenate(
            [x[b, ((g + i) % 4) * NQ:(((g + i) % 4) + 1) * NQ, :].T
             for i in range(4)], axis=1
        )
        in_maps.append({
            "xT": np.ascontiguousarray(xt_rot).astype(ml_dtypes.bfloat16),
            "w_qkv": wq_bf,
            "w_proj": wp_bf,
            "b_qk": bqk,
            "b_v": bv,
            "b_prj": bp,
        })
    return in_maps


def run(inputs, trace=False):
    global _compiled
    if _compiled is None:
        _compiled = build()
    in_maps = make_in_maps(**inputs)
    res = bass_utils.run_bass_kernel_spmd(
        _compiled, in_maps, core_ids=list(range(NC)), trace=trace
    )
    full = np.empty((B, N, D), dtype=np.float32)
    for c in range(NC):
        b, g = divmod(c, 4)
        full[b, g * NQ:(g + 1) * NQ, :] = res.results[c]["out"]
    return full, res


def kernel(x, W_qkv, b_qkv, W_proj, b_proj):
    full, _ = run(dict(x=x, W_qkv=W_qkv, b_qkv=b_qkv, W_proj=W_proj, b_proj=b_proj))
    return full


# revision 13
# speedup vs baseline: 1.0695x; 1.0695x over previous
"""Trainium2 8-core attention kernel (B=2, N=2048, D=1024, H=16).

Sharding: core c = 4*b + g handles batch b, query rows [g*512, (g+1)*512),
all 16 heads. Each core receives the full x^T of its batch with sequence
blocks rotated so its own block sits at column 0 (keys are permutation-
invariant under softmax). Heads 0-3 compute K/V locally over the whole
sequence; the remaining heads' K/V shards AllGather in three 4-rank
chunks that overlap attention.

Schedule (v2): DMA streams are k-interleaved and stage-1 runs k-outer
across PSUM banks so the PE starts as soon as the first x^T/W tiles
land. V for m-blocks 0-3 is computed over all 8 duos at once (N=1024),
which both feeds the local duos' first k-tiles and stages every AG
chunk's V payload up front - all three AllGathers are in flight within
a few microseconds of each other, long before their consumer duos.
Leftover stage-1 work (V m4-15 for the local duos, chunk K computes,
Q tiles 4-7, W_proj loads) is emitted as fillers inside the exp-paced
attention window. Attention runs in transposed-score orientation with
row-tiled concurrent score matmuls and a ones-column on V accumulating
softmax denominators in PSUM row 64. Normalization is PE-free:
reciprocal_approx_fast (DVE) + partition_broadcast (GpSimd) + one DVE
multiply. The projection's first PSUM groups pre-run their k<7 steps
inside duo 7's window; output stores stream per block.
"""

import sys

if "/opt/trn_rl_repo" not in sys.path:
    sys.path.insert(0, "/opt/trn_rl_repo")

import numpy as np
import ml_dtypes

import concourse.bass as bass
import concourse.mybir as mybir
from concourse import bacc, tile
from concourse import bass_utils

FP32 = mybir.dt.float32
BF16 = mybir.dt.bfloat16

B, N, D = 2, 2048, 1024
H, HD = 16, 64
SCALE = HD ** -0.5
NC = 8
GROUPS = [[0, 1, 2, 3], [4, 5, 6, 7]]
NQ = N // 4          # query rows per core (512)
KT = N // 128        # key k-tiles (16)
CT = D // 128        # 128-channel tiles per D (8)
LOCAL_DUOS = 4                       # duos computed locally over full seq
CHUNKS = [2, 1, 1]                   # AllGather chunks (duos), for duos 4..7
DUO_ELEMS = 128 * NQ + 2 * NQ * HD   # per-duo: 1 K^T pair + 2 V heads
CH_OFF = [sum(CHUNKS[:i]) for i in range(len(CHUNKS) + 1)]  # AG duo offsets

_compiled = None


def build():
    from contextlib import ExitStack

    nc = bacc.Bacc("TRN2", target_bir_lowering=False, debug=False, num_devices=NC)

    xT = nc.dram_tensor("xT", [D, N], BF16, kind="ExternalInput")
    w_qkv = nc.dram_tensor("w_qkv", [D, 3 * D], BF16, kind="ExternalInput")
    w_proj = nc.dram_tensor("w_proj", [D, D], BF16, kind="ExternalInput")
    b_qk = nc.dram_tensor("b_qk", [128, 16], FP32, kind="ExternalInput")
    b_v = nc.dram_tensor("b_v", [128, D], FP32, kind="ExternalInput")
    b_prj = nc.dram_tensor("b_prj", [128, D], FP32, kind="ExternalInput")
    out = nc.dram_tensor("out", [NQ, D], FP32, kind="ExternalOutput")

    with tile.TileContext(nc) as tc, ExitStack() as ctx:
        wqk_pool = ctx.enter_context(tc.tile_pool(name="wqk", bufs=16))
        wv_pool = ctx.enter_context(tc.tile_pool(name="wv", bufs=8))
        wp_pool = ctx.enter_context(tc.tile_pool(name="wp", bufs=8))
        xt_pool = ctx.enter_context(tc.tile_pool(name="xt", bufs=8))
        qt_pool = ctx.enter_context(tc.tile_pool(name="qt", bufs=8))
        bias_pool = ctx.enter_context(tc.tile_pool(name="bias", bufs=3))
        stg_pool = ctx.enter_context(tc.tile_pool(name="stg", bufs=3))
        vstg_pool = ctx.enter_context(tc.tile_pool(name="vstg", bufs=3))
        ktp_pool = ctx.enter_context(tc.tile_pool(name="ktp", bufs=4))
        vsb_pool = ctx.enter_context(tc.tile_pool(name="vsb", bufs=4))
        es_pool = ctx.enter_context(tc.tile_pool(name="es", bufs=5))
        ot_pool = ctx.enter_context(tc.tile_pool(name="ot", bufs=8))
        nrm_pool = ctx.enter_context(tc.tile_pool(name="nrm", bufs=3))
        y_pool = ctx.enter_context(tc.tile_pool(name="yy", bufs=2))
        # PSUM: psA = 2 x [128,1024] (4 banks), psB = 4 x [128,512] (4 banks)
        psA = ctx.enter_context(tc.tile_pool(name="psA", bufs=2, space="PSUM"))
        psB = ctx.enter_context(tc.tile_pool(name="psB", bufs=4, space="PSUM"))
        dram = ctx.enter_context(tc.tile_pool(name="dram", bufs=1, space="DRAM"))

        # ---- DMA streams: xt[k] + K-weight[k] interleaved, then Q-half0,
        # ---- then V weights, then Q-half1 ----
        xt = []
        wqk_k = []
        for k in range(CT):
            t = xt_pool.tile([128, N], BF16, tag="xt", name=f"xt{k}")
            nc.scalar.dma_start(t[:], xT.ap()[k * 128:(k + 1) * 128, :])
            xt.append(t)
            t = wqk_pool.tile([128, D], BF16, tag="wqk", name=f"wqkK{k}")
            nc.sync.dma_start(t[:], w_qkv.ap()[k * 128:(k + 1) * 128, D:2 * D])
            wqk_k.append(t)

        bqk_sb = bias_pool.tile([128, 16], FP32, tag="bias")
        nc.scalar.dma_start(bqk_sb[:], b_qk.ap()[:])
        bv_sb = bias_pool.tile([128, D], FP32, tag="bias")
        nc.scalar.dma_start(bv_sb[:], b_v.ap()[:])

        wqk_q = []
        for k in range(CT):
            t = wqk_pool.tile([128, D], BF16, tag="wqk", name=f"wqkQ{k}")
            nc.gpsimd.dma_start(t[:, 0:512], w_qkv.ap()[k * 128:(k + 1) * 128, 0:512])
            wqk_q.append(t)
        wv_t = []
        for k in range(CT):
            t = wv_pool.tile([128, D], BF16, tag="wv", name=f"wv{k}")
            nc.sync.dma_start(t[:], w_qkv.ap()[k * 128:(k + 1) * 128, 2 * D:3 * D])
            wv_t.append(t)
        for k in range(CT):
            nc.gpsimd.dma_start(
                wqk_q[k][:, 512:1024],
                w_qkv.ap()[k * 128:(k + 1) * 128, 512:1024],
            )

        # ---- DRAM bounce + AG buffers (layout identical to baseline) ----
        TOT = CH_OFF[-1] * DUO_ELEMS
        kv_in = dram.tile([TOT], BF16, tag="kvin")
        kv_ag = dram.tile([4 * TOT], BF16, tag="kvag")

        duo_tiles = {}

        # ---- phase 1: K^T for local duos 0,1 - k-outer over 8 PSUM groups
        # (psA tile halves hold (duo, r01) pairs; psB tiles hold r23) ----
        psK = {}
        for dd in range(2):
            a = psA.tile([128, 2 * NQ], FP32, tag="sq", name=f"psKa{dd}")
            b0 = psB.tile([128, NQ], FP32, tag="acc", name=f"psKb{dd}0")
            b1 = psB.tile([128, NQ], FP32, tag="acc", name=f"psKb{dd}1")
            psK[dd] = [a[:, 0:NQ], a[:, NQ:2 * NQ], b0[:], b1[:]]
        for k in range(CT):
            for dd in range(2):
                for r in range(4):
                    nc.tensor.matmul(
                        psK[dd][r], wqk_k[k][:, dd * 128:(dd + 1) * 128],
                        xt[k][:, r * NQ:(r + 1) * NQ],
                        start=(k == 0), stop=(k == CT - 1),
                    )
        ktp01 = []
        for dd in range(2):
            ktp = ktp_pool.tile([128, N], BF16, tag="ktp", name=f"ktpL{dd}")
            for r in range(4):
                nc.vector.tensor_scalar_add(
                    ktp[:, r * NQ:(r + 1) * NQ], psK[dd][r],
                    bqk_sb[:, 8 + dd:9 + dd],
                )
            ktp01.append(ktp)

        # local V tiles for duos 0-3 (filled progressively below)
        va_loc = []
        for dd in range(LOCAL_DUOS):
            va = vsb_pool.tile([128, KT * 2 * (HD + 1)], BF16, tag="vsb",
                               name=f"vaL{dd}")
            nc.vector.memset(
                va[:].rearrange("cc (tj ef) -> cc tj ef", ef=HD + 1)[:, :, HD:HD + 1],
                1.0,
            )
            va_loc.append(va)
            if dd < 2:
                duo_tiles[dd] = (ktp01[dd], va)

        # ---- phase 2: Q tiles 0-3 (psB) and V m0-1 over all duos (psA),
        # k-outer; then V m2-3, Q happens first so scores can start ----
        qt = [None] * CT

        def emit_q(t, ps):
            for k in range(CT):
                nc.tensor.matmul(
                    ps[:], wqk_q[k][:, t * 128:(t + 1) * 128], xt[k][:, 0:NQ],
                    start=(k == 0), stop=(k == CT - 1),
                )
            sb = qt_pool.tile([128, NQ], BF16, tag="qt", name=f"qt{t}")
            nc.vector.tensor_scalar_add(sb[:], ps[:], bqk_sb[:, t:t + 1])
            qt[t] = sb

        # V staging tiles per chunk: [p, m(4), h(2*nduo), e(64)]
        sbv_c = []
        for c, nduo in enumerate(CHUNKS):
            sbv_c.append(vstg_pool.tile([128, 4 * 128 * nduo], BF16, tag="vstg",
                                        name=f"vchunk{c}"))

        def drain_vm(m, ps):
            # ps: [128, 1024] = V for all 8 duos, m-block m. Local va writes
            # for duos 0,1 plus chunk staging for duos 2-7.
            for dd in range(LOCAL_DUOS):
                va4 = va_loc[dd][:].rearrange(
                    "cc (t j ef) -> cc t j ef", t=KT, j=2, ef=HD + 1
                )
                nc.vector.scalar_tensor_tensor(
                    va4[:, m, :, 0:HD],
                    ps[:].rearrange("p (h e) -> p h e", e=HD)[:, 2 * dd:2 * dd + 2, :],
                    0.0,
                    bv_sb[:].rearrange("p (h e) -> p h e", e=HD)[:, 2 * dd:2 * dd + 2, :],
                    op0=mybir.AluOpType.bypass, op1=mybir.AluOpType.add,
                )
            for c, nduo in enumerate(CHUNKS):
                d0 = LOCAL_DUOS + CH_OFF[c]
                nc.vector.scalar_tensor_tensor(
                    sbv_c[c][:, m * 128 * nduo:(m + 1) * 128 * nduo],
                    ps[:, d0 * 128:(d0 + nduo) * 128], 0.0,
                    bv_sb[:, d0 * 128:(d0 + nduo) * 128],
                    op0=mybir.AluOpType.bypass, op1=mybir.AluOpType.add,
                )

        # Q tiles 0,1 first (scores for duo 0/1), then V m0-1 k-outer
        qps = [psB.tile([128, NQ], FP32, tag="acc", name=f"psQ{t}")
               for t in range(2)]
        vps = [psA.tile([128, 2 * NQ], FP32, tag="sq", name=f"psVm{m}")
               for m in range(2)]
        for k in range(CT):
            for t in range(2):
                nc.tensor.matmul(
                    qps[t][:], wqk_q[k][:, t * 128:(t + 1) * 128], xt[k][:, 0:NQ],
                    start=(k == 0), stop=(k == CT - 1),
                )
            for m in range(2):
                for half in range(2):
                    nc.tensor.matmul(
                        vps[m][:, half * NQ:(half + 1) * NQ],
                        xt[k][:, m * 128:(m + 1) * 128],
                        wv_t[k][:, half * NQ:(half + 1) * NQ],
                        start=(k == 0), stop=(k == CT - 1),
                    )
        for t in range(2):
            sb = qt_pool.tile([128, NQ], BF16, tag="qt", name=f"qt{t}")
            nc.vector.tensor_scalar_add(sb[:], qps[t][:], bqk_sb[:, t:t + 1])
            qt[t] = sb
        for m in range(2):
            drain_vm(m, vps[m])

        # V m2-3 (k-inner; all wv present by now)
        for m in range(2, 4):
            ps = psA.tile([128, 2 * NQ], FP32, tag="sq", name=f"psVm{m}")
            for k in range(CT):
                for half in range(2):
                    nc.tensor.matmul(
                        ps[:, half * NQ:(half + 1) * NQ],
                        xt[k][:, m * 128:(m + 1) * 128],
                        wv_t[k][:, half * NQ:(half + 1) * NQ],
                        start=(k == 0), stop=(k == CT - 1),
                    )
            drain_vm(m, ps)

        # ---- chunk K computes + AG launches (chunk 0 now; 1,2 as fillers) ----
        def emit_chunk_k(c):
            nduo = CHUNKS[c]
            base = CH_OFF[c] * DUO_ELEMS
            d0 = LOCAL_DUOS + CH_OFF[c]
            ksz = nduo * 128 * NQ
            kin = kv_in[base:base + ksz].rearrange("(p q) -> p q", q=NQ)
            vin = kv_in[base + ksz:base + nduo * DUO_ELEMS].rearrange(
                "(p m h e) -> p m h e", p=128, m=4, h=2 * nduo, e=HD
            )
            for tt in range(nduo):
                d = d0 + tt
                ps = psB.tile([128, NQ], FP32, tag="acc", name=f"psKc{c}{tt}")
                for k in range(CT):
                    nc.tensor.matmul(
                        ps[:], wqk_k[k][:, d * 128:(d + 1) * 128],
                        xt[k][:, 0:NQ],
                        start=(k == 0), stop=(k == CT - 1),
                    )
                sb = stg_pool.tile([128, NQ], BF16, tag="stg", name=f"ksb{c}{tt}")
                nc.vector.tensor_scalar_add(sb[:], ps[:], bqk_sb[:, 8 + d:9 + d])
                nc.sync.dma_start(kin[tt * 128:(tt + 1) * 128, :], sb[:])
            nc.sync.dma_start(vin.rearrange("p m h e -> p (m h e)"), sbv_c[c][:])
            nc.gpsimd.collective_compute(
                "AllGather", mybir.AluOpType.bypass, replica_groups=GROUPS,
                ins=[kv_in[base:base + nduo * DUO_ELEMS].opt()],
                outs=[kv_ag[4 * base:4 * (base + nduo * DUO_ELEMS)].opt()],
            )

        def emit_chunk_loads(c):
            nduo = CHUNKS[c]
            base = CH_OFF[c] * DUO_ELEMS
            d0 = LOCAL_DUOS + CH_OFF[c]
            cbase4 = 4 * base
            blk = kv_ag[cbase4:cbase4 + 4 * nduo * DUO_ELEMS]
            for dd in range(nduo):
                d = d0 + dd
                ktp = ktp_pool.tile([128, N], BF16, tag="ktp", name=f"ktp{d}")
                nc.sync.dma_start(
                    ktp[:].rearrange("p (r q) -> p r q", r=4),
                    blk.rearrange("(r x p q) -> x p r q",
                                  r=4, x=nduo * DUO_ELEMS // (128 * NQ),
                                  p=128, q=NQ)[dd],
                )
                va = vsb_pool.tile([128, KT * 2 * (HD + 1)], BF16, tag="vsb",
                                   name=f"va{d}")
                va5 = va[:].rearrange(
                    "cc (r sh j ef) -> cc r sh j ef", r=4, sh=4, j=2, ef=HD + 1
                )
                for r in range(4):
                    src = bass.AP(
                        blk.tensor,
                        blk.offset + r * nduo * DUO_ELEMS + nduo * 128 * NQ
                        + 2 * dd * HD,
                        [[nduo * 512, 128], [128 * nduo, 4], [64, 2], [1, HD]],
                    )
                    eng = nc.gpsimd if r < 2 else nc.sync
                    eng.dma_start(va5[:, r, :, :, 0:HD], src)
                nc.vector.memset(
                    va[:].rearrange("cc (tj ef) -> cc tj ef",
                                    ef=HD + 1)[:, :, HD:HD + 1],
                    1.0,
                )
                duo_tiles[d] = (ktp, va)

        # all three AG doorbells fire before attention begins; the gathers
        # overlap the four local duos' attention
        for c in range(len(CHUNKS)):
            emit_chunk_k(c)

        # K^T for local duos 2,3 (k-inner; all inputs resident by now)
        for dd in range(2, LOCAL_DUOS):
            ktp = ktp_pool.tile([128, N], BF16, tag="ktp", name=f"ktpL{dd}")
            for r in range(4):
                ps = psB.tile([128, NQ], FP32, tag="acc", name=f"psK{dd}{r}")
                for k in range(CT):
                    nc.tensor.matmul(
                        ps[:], wqk_k[k][:, dd * 128:(dd + 1) * 128],
                        xt[k][:, r * NQ:(r + 1) * NQ],
                        start=(k == 0), stop=(k == CT - 1),
                    )
                nc.vector.tensor_scalar_add(
                    ktp[:, r * NQ:(r + 1) * NQ], ps[:], bqk_sb[:, 8 + dd:9 + dd]
                )
            duo_tiles[dd] = (ktp, va_loc[dd])

        # ---- filler units: emitted inside the attention window ----
        fillers = []

        def filler_q(t):
            def go():
                ps = psB.tile([128, NQ], FP32, tag="acc", name=f"psQ{t}")
                emit_q(t, ps)
            return go

        def filler_vloc(m):
            def go():
                ps = psB.tile([128, 2 * LOCAL_DUOS * HD], FP32, tag="acc",
                              name=f"psVL{m}")
                for k in range(CT):
                    nc.tensor.matmul(
                        ps[:], xt[k][:, m * 128:(m + 1) * 128],
                        wv_t[k][:, 0:2 * LOCAL_DUOS * HD],
                        start=(k == 0), stop=(k == CT - 1),
                    )
                for dd in range(LOCAL_DUOS):
                    va4 = va_loc[dd][:].rearrange(
                        "cc (t j ef) -> cc t j ef", t=KT, j=2, ef=HD + 1
                    )
                    nc.vector.scalar_tensor_tensor(
                        va4[:, m, :, 0:HD],
                        ps[:].rearrange("p (h e) -> p h e", e=HD)[:, 2 * dd:2 * dd + 2, :],
                        0.0,
                        bv_sb[:].rearrange("p (h e) -> p h e", e=HD)[:, 2 * dd:2 * dd + 2, :],
                        op0=mybir.AluOpType.bypass, op1=mybir.AluOpType.add,
                    )
            return go

        wp = []
        bp_sb = bias_pool.tile([128, D], FP32, tag="bias")

        def filler_wp(half):
            def go():
                if half == 0:
                    nc.sync.dma_start(bp_sb[:], b_prj.ap()[:])
                for k in range(half * 4, half * 4 + 4):
                    t = wp_pool.tile([128, D], BF16, tag="wp", name=f"wp{k}")
                    nc.sync.dma_start(t[:], w_proj.ap()[k * 128:(k + 1) * 128, :])
                    wp.append(t)
            return go

        def filler_loads(c):
            def go():
                emit_chunk_loads(c)
            return go

        # V m4-15 handled inline per-kt during duo 0; the rest as fillers.
        # Pump order: (d0,kt8) (d0,kt12) (d1,kt8) ... two slots per duo.
        fillers.extend([
            filler_q(2), filler_loads(0), filler_q(3), filler_q(4),
            filler_loads(1), filler_q(5), filler_q(6), filler_loads(2),
            filler_q(7), filler_wp(0), filler_wp(1),
        ])

        def pump(n):
            for _ in range(n):
                if fillers:
                    fillers.pop(0)()

        # ---- attention ----
        ot = []
        norm_q = []        # (ocs pair, duo) awaiting normalization

        def drain_oacc(o_acc, d):
            ocs = []
            for j in range(2):
                oc = nrm_pool.tile([HD + 1, NQ], FP32, tag="oc",
                                   name=f"oc{d}_{j}")
                nc.vector.tensor_copy(oc[:], o_acc[j][0:HD + 1, :])
                ocs.append(oc)
            return ocs

        def normalize(ocs, d):
            otd = ot_pool.tile([128, NQ], BF16, tag="ot", name=f"ot{d}")
            for j in range(2):
                # reciprocal_approx_fast mishandles base-partition-64 inputs;
                # stage the denominator row at partition 0 first.
                den0 = nrm_pool.tile([1, NQ], FP32, tag="den0", name=f"dn{d}_{j}",
                                     bufs=2)
                nc.vector.tensor_copy(den0[:], ocs[j][HD:HD + 1, :])
                rr = nrm_pool.tile([1, NQ], FP32, tag="rr", name=f"rr{d}_{j}",
                                   bufs=2)
                nc.vector.reciprocal_approx_fast(rr[:], den0[:])
                rb = nrm_pool.tile([HD, NQ], FP32, tag="rb", name=f"rb{d}_{j}",
                                   bufs=2)
                nc.gpsimd.partition_broadcast(rb[:], rr[:])
                nc.vector.scalar_tensor_tensor(
                    otd[j * HD:(j + 1) * HD, :],
                    ocs[j][0:HD, :], 0.0, rb[:],
                    op0=mybir.AluOpType.bypass, op1=mybir.AluOpType.mult,
                )
            ot.append(otd)

        for d in range(H // 2):
            ktp, va = duo_tiles[d]
            va4 = va[:].rearrange("cc (t j ef) -> cc t j ef", t=KT, j=2, ef=HD + 1)

            o_acc = [
                psB.tile([128, NQ], FP32, tag="acc", name=f"oacc{d}_{j}")
                for j in range(2)
            ]
            es_tiles = [None] * KT

            def emit_pv(kt_i):
                es_kt = es_tiles[kt_i]
                for j in range(2):
                    nc.tensor.matmul(
                        o_acc[j][0:HD + 1, :],
                        va4[:, kt_i, j, :],
                        es_kt[:, j * NQ:(j + 1) * NQ],
                        start=(kt_i == 0), stop=(kt_i == KT - 1),
                    )

            pv_lag = 3 if d == 0 else 1
            for kt in range(KT):
                # duo 0: V m-tiles 4-15 are produced just ahead of their PV
                if d == 0 and 2 <= kt < 14:
                    filler_vloc(kt + 2)()
                s = psA.tile([128, 2 * NQ], FP32, tag="sq", name=f"s{d}_{kt}")
                for i in range(2):
                    nc.tensor.matmul(
                        s[:, i * NQ:(i + 1) * NQ],
                        ktp[i * HD:(i + 1) * HD, kt * 128:(kt + 1) * 128],
                        qt[d][i * HD:(i + 1) * HD, :],
                        start=True, stop=True,
                    )
                es = es_pool.tile([128, 2 * NQ], BF16, tag="es",
                                  name=f"es{d}_{kt}")
                nc.scalar.activation(
                    es[:], s[:], mybir.ActivationFunctionType.Exp, scale=SCALE
                )
                es_tiles[kt] = es
                if kt >= pv_lag:
                    emit_pv(kt - pv_lag)
                if kt == 4 and norm_q:
                    normalize(*norm_q.pop(0))
                if kt == 8 and d <= 5:
                    pump(1)
                elif kt == 12 and d <= 5:
                    pump(1)
            for kt_i in range(KT - pv_lag, KT):
                emit_pv(kt_i)
            norm_q.append((drain_oacc(o_acc, d), d))

        pump(len(fillers))

        # ---- projection: groups 0-1 pre-run k0-6 inside duo 7 via the
        # normal emission position; after last normalize, finish all ----
        while norm_q:
            normalize(*norm_q.pop(0))

        for m in range(NQ // 128):
            for n0 in range(D // 512):
                ps = psB.tile([128, 512], FP32, tag="acc", name=f"psP{m}{n0}")
                for k in range(CT):
                    nc.tensor.matmul(
                        ps[:], ot[k][:, m * 128:(m + 1) * 128],
                        wp[k][:, n0 * 512:(n0 + 1) * 512],
                        start=(k == 0), stop=(k == CT - 1),
                    )
                y = y_pool.tile([128, 512], FP32, tag="yy", name=f"y{m}{n0}")
                nc.vector.scalar_tensor_tensor(
                    y[:], ps[:], 0.0, bp_sb[:, n0 * 512:(n0 + 1) * 512],
                    op0=mybir.AluOpType.bypass, op1=mybir.AluOpType.add,
                )
                nc.sync.dma_start(
                    out.ap()[m * 128:(m + 1) * 128, n0 * 512:(n0 + 1) * 512], y[:]
                )

    nc.compile()
    return nc


def make_in_maps(x, W_qkv, b_qkv, W_proj, b_proj):
    x = np.asarray(x, dtype=np.float32)
    W_qkv = np.asarray(W_qkv, dtype=np.float32)
    b_qkv = np.asarray(b_qkv, dtype=np.float32)
    W_proj = np.asarray(W_proj, dtype=np.float32)
    b_proj = np.asarray(b_proj, dtype=np.float32)

    wq_bf = W_qkv.astype(ml_dtypes.bfloat16)
    wp_bf = W_proj.astype(ml_dtypes.bfloat16)
    bqk = np.ascontiguousarray(b_qkv[:2 * D].reshape(16, 128).T)
    bv = np.tile(b_qkv[2 * D:], (128, 1)).astype(np.float32)
    bp = np.tile(b_proj, (128, 1)).astype(np.float32)

    in_maps = []
    for c in range(NC):
        b, g = divmod(c, 4)
        xt_rot = np.concatenate(
            [x[b, ((g + i) % 4) * NQ:(((g + i) % 4) + 1) * NQ, :].T
             for i in range(4)], axis=1
        )
        in_maps.append({
            "xT": np.ascontiguousarray(xt_rot).astype(ml_dtypes.bfloat16),
            "w_qkv": wq_bf,
            "w_proj": wp_bf,
            "b_qk": bqk,
            "b_v": bv,
            "b_prj": bp,
        })
    return in_maps


def run(inputs, trace=False):
    global _compiled
    if _compiled is None:
        _compiled = build()
    in_maps = make_in_maps(**inputs)
    res = bass_utils.run_bass_kernel_spmd(
        _compiled, in_maps, core_ids=list(range(NC)), trace=trace
    )
    full = np.empty((B, N, D), dtype=np.float32)
    for c in range(NC):
        b, g = divmod(c, 4)
        full[b, g * NQ:(g + 1) * NQ, :] = res.results[c]["out"]
    return full, res


def kernel(x, W_qkv, b_qkv, W_proj, b_proj):
    full, _ = run(dict(x=x, W_qkv=W_qkv, b_qkv=b_qkv, W_proj=W_proj, b_proj=b_proj))
    return full
